# revision 13
# baseline (speedup 1.0000x reference)
"""Causal self-attention (B=1, S=4096, E=1024, H=16, D=64) on 8 trn2 NeuronCores.

Sharding: head-parallel. Core c owns heads {2c, 2c+1}. Host sums the 8
partial [S, E] outputs (row-parallel out_proj reduce) in f32.

v3 (vs v2 @349-375us): span was 380us with no engine >61% busy —
dependency stalls, a 55us serial init, and 46us of PE transposes.
  - x is pre-transposed and cast to bf16 on the HOST (input marshalling,
    like the existing w re-slicing): kills all 256 PE x-transposes, the
    64 DVE xT casts, and halves the x DMA bytes.
  - all weights/constants arrive via DMA in their final dtype (bf16 w,
    packed bias, identity, causal-mask tiles, and the qP8/kT8/v8/v_sb
    zero/ones init patterns): the 32us of GpSimd memsets and 28us of
    DVE init casts disappear, and phase A no longer false-depends on a
    weight-staging pool (PE started 55us late in v2).
  - causal diag masking via constant mask-tile matmul accumulated into
    the scores PSUM before exp (exp(-1250)=0): removes 64 GpSimd
    affine_selects from the diag critical chain.
  - far-loop software-pipeline skew: scores(g+1) is emitted before
    AV(g) so the PE never sits behind ACT's exp round-trip.
  - first AV matmul per i-block uses start=True: kills 16 DVE memsets.
  - partial outputs written bf16 (host sums in f32): halves out DMA,
    cheaper bias-add.
"""

import numpy as np
import ml_dtypes

S = 4096
E = 1024
D = 64
N_HEAD = 16
N_CORES = 8
HL = N_HEAD // N_CORES  # heads per core = 2
CLOC = HL * D           # 128 local qkv cols per q/k/v

BF = ml_dtypes.bfloat16
F8 = ml_dtypes.float8_e4m3

_CACHE = {}


def build_nc(s=S, debug=False):
    import concourse.bacc as bacc
    import concourse.mybir as mybir
    from concourse.tile import TileContext

    f32 = mybir.dt.float32
    bf16 = mybir.dt.bfloat16
    fp8 = mybir.dt.float8e4
    DR = mybir.MatmulPerfMode.DoubleRow
    Exp = mybir.ActivationFunctionType.Exp

    n_sb = s // 512    # 512-row s-blocks (phase A granularity)
    n_ib = s // 256    # 256-query i-blocks (phase B granularity)
    n_jc = s // 128    # 128-key j-chunks
    n_g = s // 256     # 256-key far groups

    nc = bacc.Bacc()
    # xTt[sb*128+p, e*512+c] = x[sb*512+c, e*128+p]: per-sb loads are one
    # DMA of 128 contiguous rows (descriptor count is what DMA queues pay for)
    xTd = nc.declare_dram_parameter("xTt", [E, s], bf16, isOutput=False)
    xT8d = nc.declare_dram_parameter("xT8t", [E, s], fp8, isOutput=False)
    wvd = nc.declare_dram_parameter("w_vt", [CLOC, E], bf16, isOutput=False)
    w8d = nc.declare_dram_parameter("w8_qk", [128, 2048], fp8, isOutput=False)
    bqd = nc.declare_dram_parameter("bq_loc", [CLOC, 3], f32, isOutput=False)
    wod = nc.declare_dram_parameter("w_out_loc", [CLOC, E], bf16, isOutput=False)
    identd = nc.declare_dram_parameter("ident", [128, 128], bf16, isOutput=False)
    maskd = nc.declare_dram_parameter("masks", [128, 1024], bf16, isOutput=False)
    outp = nc.declare_dram_parameter("out_p", [s, E], bf16, isOutput=True)
    if debug:
        dq = nc.declare_dram_parameter("dbg_qP8", [128, n_ib * 1024], f32, isOutput=True)
        dk = nc.declare_dram_parameter("dbg_kT8", [128, n_jc * 256], f32, isOutput=True)
        dv = nc.declare_dram_parameter("dbg_v8", [128, n_g * 512], f32, isOutput=True)
        da = nc.declare_dram_parameter("dbg_attT", [128, s], f32, isOutput=True)

    with TileContext(nc) as tc, tc.tile_pool(name="persist", bufs=1) as pp:
        # ---- persistent tiles ----
        # packed q, fp8, with a zero second k-tile for DoubleRow:
        # per ib: [tile0: h0 q (cols 0:256, parts 0:64) | h1 q (cols 256:512,
        # parts 64:128), zeros elsewhere][tile1: 512 zero cols]
        qP8 = pp.tile([128, n_ib * 1024], fp8, name="qP8")
        # kT chunks fp8: per jc: [128 real cols][128 zero cols]
        kT8 = pp.tile([128, n_jc * 256], fp8, name="kT8")
        # v fp8 for far AV: per (g,h): [t0: v(64)|1|0*63][t1: v(64)|1|0*63]
        # M=128, rows 65:128 of the AV output are zeros, row 64 = denominator.
        v8 = pp.tile([128, n_g * 512], fp8, name="v8")
        # v bf16 for diagonal AV: per jc: [vh0|1][vh1|1]
        v_sb = pp.tile([128, n_jc * 130], bf16, name="v_sb")
        attT = pp.tile([128, s], bf16, name="attT")
        wv_sb = pp.tile([128, 8 * CLOC], bf16, name="wv_sb")
        w8_sb = pp.tile([128, 2048], fp8, name="w8_sb")
        wo_sb = pp.tile([128, E], bf16, name="wo_sb")
        bq_sb = pp.tile([128, 3], f32, name="bq_sb")
        id_sb = pp.tile([128, 128], bf16, name="id_sb")
        m_sb = pp.tile([128, 1024], bf16, name="m_sb")

        # ---- const loads. Queues: sync = w8 + fp8 x stream, scalar = bf16
        # x stream + v/out weights. The zero/ones init runs as gpsimd
        # MEMSETs: that engine is otherwise idle, so ~33us of memset
        # executes in parallel with the DMA streams and first proj.
        # qP8's zeroing covers only the never-written regions (tile1 +
        # the off-diagonal tile0 blocks) so phase A's q writes don't
        # queue behind it.
        nc.sync.dma_start(w8_sb[:], w8d[:, :])
        nc.sync.dma_start(bq_sb[:], bqd[:, :])
        nc.scalar.dma_start(wv_sb[:], wvd[:, :])
        nc.scalar.dma_start(id_sb[:], identd[:, :])
        nc.scalar.dma_start(m_sb[:], maskd[:, :])
        nc.scalar.dma_start(wo_sb[:], wod[:, :])
        nc.gpsimd.memset(kT8[:], 0.0)
        nc.gpsimd.memset(v_sb[:], 1.0)
        nc.gpsimd.memset(v8[:], 0.0)
        nc.gpsimd.memset(
            v8.rearrange("p (a c) -> p a c", c=128)[:, :, 64:65], 1.0)
        qP8_b = qP8.rearrange("p (b c) -> p b c", c=1024)
        nc.gpsimd.memset(qP8_b[:, :, 512:1024], 0.0)
        nc.gpsimd.memset(qP8_b[0:64, :, 256:512], 0.0)
        nc.gpsimd.memset(qP8_b[64:128, :, 0:256], 0.0)

        # pools (PSUM: 1 + 2 + 4 + 1 = 8 banks)
        with tc.tile_pool(name="paxt", bufs=2) as paxt, \
             tc.tile_pool(name="pa", bufs=2) as pa, \
             tc.tile_pool(name="ptr", bufs=1, space="PSUM") as ptr, \
             tc.tile_pool(name="pmm", bufs=2, space="PSUM") as pmm, \
             tc.tile_pool(name="psc", bufs=2, space="PSUM") as psc, \
             tc.tile_pool(name="pot", bufs=1, space="PSUM") as pot, \
             tc.tile_pool(name="pbw", bufs=4) as pbw, \
             tc.tile_pool(name="pbwd", bufs=2) as pbwd, \
             tc.tile_pool(name="pbn", bufs=2) as pbn, \
             tc.tile_pool(name="pc", bufs=3) as pc:

            b_queue = []

            def pump_b(k):
                for _ in range(min(k, len(b_queue))):
                    b_queue.pop(0)()

            # ---------------- phase B pieces ----------------
            def qP8_ap(ib):
                return qP8[:, ib * 1024:(ib + 1) * 1024].rearrange(
                    "p (t c) -> p t c", t=2)

            def kT8_ap(jc):
                return kT8[:, jc * 256:(jc + 1) * 256].rearrange(
                    "p (t c) -> p t c", t=2)

            def queue_b(ib):
                st = {}

                def get_ot():
                    if "ot" not in st:
                        st["ot"] = pot.tile([128, 512], f32, tag="ot", name="ot")
                    return st["ot"]

                def t_far_sc(g):
                    scp = psc.tile([128, 1024], f32, tag="sc")
                    for t in range(2):
                        nc.tensor.matmul(
                            scp[:, t * 512:(t + 1) * 512],
                            kT8_ap(2 * g + t), qP8_ap(ib),
                            start=True, stop=True, perf_mode=DR,
                        )
                    # wt8 h-major: [h0: t0(256) t1(256) | h1: t0 t1] so the
                    # AV rhs [2, 256] is contiguous (dual-fp8 ISA rule)
                    wt8 = pbw.tile([128, 1024], fp8, tag="wt8")
                    nc.scalar.activation(
                        wt8.rearrange("p (h t c) -> p t h c", h=2, t=2),
                        scp[:].rearrange("p (t h c) -> p t h c", t=2, h=2),
                        Exp, scale=0.125)
                    st[g] = wt8

                def t_far_av(g, first):
                    wt8 = st.pop(g)
                    ot = get_ot()
                    for h in range(2):
                        nc.tensor.matmul(
                            ot[:, h * 256:(h + 1) * 256],
                            v8[:, g * 512 + h * 256:
                               g * 512 + (h + 1) * 256].rearrange(
                                "p (t c) -> p t c", t=2),
                            wt8[:, h * 512:(h + 1) * 512].rearrange(
                                "p (t c) -> p t c", t=2),
                            start=(first and h == 0), stop=False, perf_mode=DR,
                            skip_group_check=True,
                        )

                def t_diag_sc():
                    scp = psc.tile([128, 1024], f32, tag="sc")
                    for t in range(2):
                        nc.tensor.matmul(
                            scp[:, t * 512:(t + 1) * 512],
                            kT8_ap(2 * ib + t), qP8_ap(ib),
                            start=True, stop=False, perf_mode=DR,
                        )
                        # accumulate the causal mask (-1e4 on masked) so
                        # exp(0.125*(s-240)) ~ 1e-13 ~ 0 (stay inside the ACT Exp table domain)
                        nc.tensor.matmul(
                            scp[:, t * 512:(t + 1) * 512],
                            id_sb[:],
                            m_sb[:, t * 512:(t + 1) * 512],
                            start=False, stop=True, skip_group_check=True,
                        )
                    wt_d = pbwd.tile([128, 1024], bf16, tag="wtd")
                    nc.scalar.activation(wt_d[:], scp[:], Exp, scale=0.125)
                    st["d"] = wt_d

                def t_diag_av(first):
                    wt_d = st.pop("d")
                    ot = get_ot()
                    for t in range(2):
                        jc = 2 * ib + t
                        for h in range(2):
                            nc.tensor.matmul(
                                ot[0:65, h * 256:(h + 1) * 256],
                                v_sb[:, jc * 130 + h * 65:
                                     jc * 130 + (h + 1) * 65],
                                wt_d[:, t * 512 + h * 256:
                                     t * 512 + (h + 1) * 256],
                                start=(first and t == 0 and h == 0),
                                stop=(t == 1 and h == 1),
                                skip_group_check=True,
                            )

                def t_norm():
                    ot = st.pop("ot")
                    onum = pbn.tile([65, 512], f32, tag="onum")
                    nc.vector.tensor_copy(onum[:], ot[0:65, :])
                    # reciprocal_approx_fast (custom DVE op) mishandles a
                    # partition-shifted input on HW: stage the denominator
                    # row to partition 0 first with a plain copy.
                    den = pbn.tile([1, 512], f32, tag="den")
                    nc.vector.tensor_copy(den[0:1, :], onum[64:65, :])
                    rcp = pbn.tile([1, 512], f32, tag="rcp")
                    nc.vector.reciprocal_approx_fast(
                        out=rcp[0:1, :], in_=den[0:1, :])
                    rb = pbn.tile([64, 512], f32, tag="rb")
                    nc.gpsimd.partition_broadcast(rb[:], rcp[0:1, :])
                    for h in range(2):
                        nc.vector.tensor_mul(
                            attT[h * 64:(h + 1) * 64,
                                 ib * 256:(ib + 1) * 256],
                            onum[0:64, h * 256:(h + 1) * 256],
                            rb[:, h * 256:(h + 1) * 256],
                        )

                # software-pipelined: sc(g+1) emitted before av(g) so the
                # PE stays a stage ahead of ACT's exp
                scs = [lambda g=g: t_far_sc(g) for g in range(ib)] + [t_diag_sc]
                avs = [lambda g=g, f=(g == 0): t_far_av(g, f)
                       for g in range(ib)] + [lambda: t_diag_av(ib == 0)]
                thunks = [scs[0]]
                for i in range(1, len(scs)):
                    thunks.append(scs[i])
                    thunks.append(avs[i - 1])
                thunks.append(avs[-1])
                thunks.append(t_norm)
                if ib > 0:
                    thunks.insert(min(2, len(thunks) - 1),
                                  lambda: emit_out_proj(ib - 1, (0,)))
                    thunks.insert(min(5, len(thunks) - 1),
                                  lambda: emit_out_proj(ib - 1, (1,)))
                b_queue.extend(thunks)

            def emit_out_proj(ib, sis=(0, 1)):
                for si in sis:
                    sb2 = 2 * ib + si
                    op = psc.tile([128, 1024], f32, tag="sc")
                    for nh2 in range(2):
                        nc.tensor.matmul(
                            op[:, nh2 * 512:(nh2 + 1) * 512],
                            attT[:, sb2 * 128:(sb2 + 1) * 128],
                            wo_sb[:, nh2 * 512:(nh2 + 1) * 512],
                            start=True, stop=True,
                        )
                    osb = pc.tile([128, 1024], bf16, tag="osb")
                    nc.vector.tensor_copy(osb[:], op[:])
                    nc.gpsimd.dma_start(
                        outp[sb2 * 128:(sb2 + 1) * 128, :], osb[:])

            # ---------------- phase A (interleaved with B) ----------------
            def xT_dma(sb):
                xt = paxt.tile([128, 8 * 512], bf16, tag="xT")
                xt8 = paxt.tile([128, 8 * 512], fp8, tag="xT8")
                nc.sync.dma_start(xt8[:], xT8d[sb * 128:(sb + 1) * 128, :])
                nc.scalar.dma_start(xt[:], xTd[sb * 128:(sb + 1) * 128, :])
                return xt, xt8

            xt_next = xT_dma(0)
            for sb in range(n_sb):
                xT_sb, xT8_sb = xt_next
                if sb + 1 < n_sb:
                    xt_next = xT_dma(sb + 1)
                q = sb + 2  # pump quantum: matches thunk production rate
                vT_t = None
                for t in range(3):
                    mmp = pmm.tile([128, 512], f32, tag="mm")
                    if t < 2:
                        # q/k proj in fp8 DoubleRow: K=256 per pass, so 4
                        # matmuls instead of 8 (cols stream at 1/cyc
                        # regardless of dtype; DR halves the pass count)
                        for pr in range(4):
                            nc.tensor.matmul(
                                mmp[:],
                                w8_sb[:, t * 1024 + pr * 256:
                                      t * 1024 + (pr + 1) * 256].rearrange(
                                    "p (u c) -> p u c", u=2),
                                xT8_sb[:, pr * 1024:(pr + 1) * 1024].rearrange(
                                    "p (u c) -> p u c", u=2),
                                start=(pr == 0), stop=(pr == 3),
                                perf_mode=DR,
                            )
                    else:
                        for ec in range(8):
                            nc.tensor.matmul(
                                mmp[:],
                                wv_sb[:, ec * 128:(ec + 1) * 128],
                                xT_sb[:, ec * 512:(ec + 1) * 512],
                                start=(ec == 0), stop=(ec == 7),
                            )
                    if t == 0:
                        # packed q -> qP8 tile0 halves (fp8), + bias
                        qP8_v = qP8.rearrange("p (b c) -> p b c", c=1024)
                        for h in range(2):
                            dst = qP8_v[h * 64:(h + 1) * 64,
                                        2 * sb:2 * sb + 2,
                                        h * 256:(h + 1) * 256]
                            src = mmp[h * 64:(h + 1) * 64, :].rearrange(
                                "p (b c) -> p b c", c=256)
                            nc.vector.tensor_scalar_add(
                                dst, src, bq_sb[h * 64:(h + 1) * 64, 0:1])
                    elif t == 1:
                        kT8_v = kT8.rearrange("p (b c) -> p b c", c=256)
                        dst = kT8_v[:, 4 * sb:4 * sb + 4, 0:128]
                        src = mmp[:].rearrange("p (b c) -> p b c", c=128)
                        nc.vector.tensor_scalar_add(dst, src, bq_sb[:, 1:2])
                    else:
                        vT_t = pa.tile([128, 512], bf16, tag="vT")
                        nc.vector.tensor_scalar_add(
                            vT_t[:], mmp[:], bq_sb[:, 2:3])
                    pump_b(q)
                for stt in range(4):
                    jc = sb * 4 + stt
                    trv = ptr.tile([128, 128], bf16, tag="tr")
                    nc.tensor.transpose(
                        trv[:],
                        vT_t[:, stt * 128:(stt + 1) * 128],
                        id_sb[:],
                    )
                    g, tt = jc // 2, jc % 2
                    dstd = v_sb[:, jc * 130:(jc + 1) * 130].rearrange(
                        "p (h c) -> p h c", h=2)[:, :, 0:64]
                    src = trv[:].rearrange("p (h c) -> p h c", h=2)
                    nc.vector.tensor_copy(dstd, src)
                    dst8 = v8.rearrange(
                        "p (g h t c) -> p g h t c", h=2, t=2, c=128
                    )[:, g, :, tt, 0:64]
                    nc.vector.tensor_copy(dst8, src)
                    pump_b(q)
                queue_b(2 * sb)
                queue_b(2 * sb + 1)
                pump_b(q)
            while b_queue:
                pump_b(1)
            emit_out_proj(n_ib - 1)
            if debug:
                with tc.tile_pool(name="dbg", bufs=2) as dp:
                    def dump(dst, srct, width, tag):
                        cw = min(2048, width)
                        for c0 in range(0, width, cw):
                            t = dp.tile([128, cw], f32, tag=tag)
                            nc.vector.tensor_copy(t[:], srct[:, c0:c0 + cw])
                            nc.sync.dma_start(dst[:, c0:c0 + cw], t[:])
                    dump(dq, qP8, n_ib * 1024, "d1")
                    dump(dk, kT8, n_jc * 256, "d2")
                    dump(dv, v8, n_g * 512, "d3")
                    dump(da, attT, s, "d4")

    nc.compile()
    return nc


def make_in_maps(x, w_qkv, b_qkv, w_out, b_out, s=S):
    x = np.asarray(x, dtype=np.float32).reshape(s, E)
    w_qkv = np.asarray(w_qkv, dtype=np.float32)
    b_qkv = np.asarray(b_qkv, dtype=np.float32)
    w_out = np.asarray(w_out, dtype=np.float32)
    b_out = np.asarray(b_out, dtype=np.float32)

    # xTt[sb*128+p, e*512+c] = x[sb*512+c, e*128+p]
    x4 = x.reshape(8, 512, 8, 128).transpose(0, 3, 2, 1)  # [sb, p, e, c]
    xTt = np.ascontiguousarray(x4.reshape(1024, 4096))
    xT = xTt.astype(BF)
    xT8 = xTt.astype(F8)
    ident = np.eye(128, dtype=BF)

    # mask tiles: m_t[k, h*256+qq] = 0 if qq >= 128*t + k else -1e4
    qq = np.arange(256)[None, :]
    kk = np.arange(128)[:, None]
    m0 = np.where(qq >= kk, 0.0, -240.0).astype(np.float32)
    m1 = np.where(qq >= 128 + kk, 0.0, -240.0).astype(np.float32)
    masks = np.concatenate([m0, m0, m1, m1], axis=1).astype(BF)

    in_maps = []
    for c in range(N_CORES):
        lo = c * CLOC
        # w8_qk[p, t*1024 + pair*256 + tile*128 + m]
        #   = w_qkv[(2*pair+tile)*128 + p, t*E + lo + m]   (fp8, DR layout)
        w8 = np.zeros((128, 2048), np.float32)
        for t in range(2):
            wt = w_qkv[:, t * E + lo:t * E + lo + CLOC]  # [E, 128]
            for pair in range(4):
                for tl in range(2):
                    src_rows = wt[(2 * pair + tl) * 128:
                                  (2 * pair + tl + 1) * 128, :]
                    w8[:, t * 1024 + pair * 256 + tl * 128:
                       t * 1024 + pair * 256 + (tl + 1) * 128] = src_rows
        w8 = w8.astype(F8)
        # w_vt[p, e*128+m] = w_qkv[e*128+p, 2E+lo+m]
        wv_loc = np.ascontiguousarray(
            w_qkv[:, 2 * E + lo:2 * E + lo + CLOC].reshape(8, 128, 128)
            .transpose(1, 0, 2).reshape(128, E)).astype(BF)
        bq_loc = np.ascontiguousarray(np.stack(
            [b_qkv[lo:lo + CLOC],
             b_qkv[E + lo:E + lo + CLOC],
             b_qkv[2 * E + lo:2 * E + lo + CLOC]], axis=1)).astype(np.float32)
        in_maps.append({
            "xTt": xT,
            "xT8t": xT8,
            "w_vt": wv_loc,
            "w8_qk": w8,
            "bq_loc": bq_loc,
            "w_out_loc": np.ascontiguousarray(
                w_out[lo:lo + CLOC, :]).astype(BF),
            "ident": ident,
            "masks": masks,
        })
    return in_maps


def kernel(x, w_qkv, b_qkv, w_out, b_out, trace=False):
    from concourse.bass_utils import run_bass_kernel_spmd

    if "nc" not in _CACHE:
        _CACHE["nc"] = build_nc()
    nc = _CACHE["nc"]
    in_maps = make_in_maps(x, w_qkv, b_qkv, w_out, b_out)
    last_err = None
    for _attempt in range(2):
        try:
            res = run_bass_kernel_spmd(nc, in_maps, list(range(N_CORES)), trace=trace)
            break
        except Exception as e:  # transient NRT device errors: retry once
            last_err = e
    else:
        raise last_err
    out = np.zeros((S, E), dtype=np.float32)
    for c in range(N_CORES):
        out += np.asarray(res.results[c]["out_p"]).astype(np.float32)
    out += np.asarray(b_out, dtype=np.float32).reshape(1, E)
    _CACHE["last_result"] = res
    return out.reshape(1, S, E)


# revision 14
# speedup vs baseline: 1.0114x; 1.0114x over previous
"""Causal self-attention (B=1, S=4096, E=1024, H=16, D=64) on 8 trn2 NeuronCores.

Sharding: head-parallel. Core c owns heads {2c, 2c+1}. Host sums the 8
partial [S, E] outputs (row-parallel out_proj reduce) in f32.

v3 (vs v2 @349-375us): span was 380us with no engine >61% busy —
dependency stalls, a 55us serial init, and 46us of PE transposes.
  - x is pre-transposed and cast to bf16 on the HOST (input marshalling,
    like the existing w re-slicing): kills all 256 PE x-transposes, the
    64 DVE xT casts, and halves the x DMA bytes.
  - all weights/constants arrive via DMA in their final dtype (bf16 w,
    packed bias, identity, causal-mask tiles, and the qP8/kT8/v8/v_sb
    zero/ones init patterns): the 32us of GpSimd memsets and 28us of
    DVE init casts disappear, and phase A no longer false-depends on a
    weight-staging pool (PE started 55us late in v2).
  - causal diag masking via constant mask-tile matmul accumulated into
    the scores PSUM before exp (exp(-1250)=0): removes 64 GpSimd
    affine_selects from the diag critical chain.
  - far-loop software-pipeline skew: scores(g+1) is emitted before
    AV(g) so the PE never sits behind ACT's exp round-trip.
  - first AV matmul per i-block uses start=True: kills 16 DVE memsets.
  - partial outputs written bf16 (host sums in f32): halves out DMA,
    cheaper bias-add.
"""

import numpy as np
import ml_dtypes

S = 4096
E = 1024
D = 64
N_HEAD = 16
N_CORES = 8
HL = N_HEAD // N_CORES  # heads per core = 2
CLOC = HL * D           # 128 local qkv cols per q/k/v

BF = ml_dtypes.bfloat16
F8 = ml_dtypes.float8_e4m3

_CACHE = {}


def build_nc(s=S, debug=False):
    import concourse.bacc as bacc
    import concourse.mybir as mybir
    from concourse.tile import TileContext

    f32 = mybir.dt.float32
    bf16 = mybir.dt.bfloat16
    fp8 = mybir.dt.float8e4
    DR = mybir.MatmulPerfMode.DoubleRow
    Exp = mybir.ActivationFunctionType.Exp

    n_sb = s // 512    # 512-row s-blocks (phase A granularity)
    n_ib = s // 256    # 256-query i-blocks (phase B granularity)
    n_jc = s // 128    # 128-key j-chunks
    n_g = s // 256     # 256-key far groups

    nc = bacc.Bacc()
    # xTt[sb*128+p, e*512+c] = x[sb*512+c, e*128+p]: per-sb loads are one
    # DMA of 128 contiguous rows (descriptor count is what DMA queues pay for)
    xTd = nc.declare_dram_parameter("xTt", [E, s], bf16, isOutput=False)
    xT8d = nc.declare_dram_parameter("xT8t", [E, s], fp8, isOutput=False)
    wvd = nc.declare_dram_parameter("w_vt", [CLOC, E], bf16, isOutput=False)
    w8d = nc.declare_dram_parameter("w8_qk", [128, 2048], fp8, isOutput=False)
    bqd = nc.declare_dram_parameter("bq_loc", [CLOC, 3], f32, isOutput=False)
    wod = nc.declare_dram_parameter("w_out_loc", [CLOC, E], bf16, isOutput=False)
    identd = nc.declare_dram_parameter("ident", [128, 128], bf16, isOutput=False)
    maskd = nc.declare_dram_parameter("masks", [128, 1024], bf16, isOutput=False)
    outp = nc.declare_dram_parameter("out_p", [s, E], bf16, isOutput=True)
    if debug:
        dq = nc.declare_dram_parameter("dbg_qP8", [128, n_ib * 1024], f32, isOutput=True)
        dk = nc.declare_dram_parameter("dbg_kT8", [128, n_jc * 256], f32, isOutput=True)
        dv = nc.declare_dram_parameter("dbg_v8", [128, n_g * 512], f32, isOutput=True)
        da = nc.declare_dram_parameter("dbg_attT", [128, s], f32, isOutput=True)

    with TileContext(nc) as tc, tc.tile_pool(name="persist", bufs=1) as pp:
        # ---- persistent tiles ----
        # packed q, fp8, with a zero second k-tile for DoubleRow:
        # per ib: [tile0: h0 q (cols 0:256, parts 0:64) | h1 q (cols 256:512,
        # parts 64:128), zeros elsewhere][tile1: 512 zero cols]
        qP8 = pp.tile([128, n_ib * 1024], fp8, name="qP8")
        # kT chunks fp8: per jc: [128 real cols][128 zero cols]
        kT8 = pp.tile([128, n_jc * 256], fp8, name="kT8")
        # v fp8 for far AV: per (g,h): [t0: v(64)|1|0*63][t1: v(64)|1|0*63]
        # M=128, rows 65:128 of the AV output are zeros, row 64 = denominator.
        v8 = pp.tile([128, n_g * 512], fp8, name="v8")
        # v bf16 for diagonal AV: per jc: [vh0|1][vh1|1]
        v_sb = pp.tile([128, n_jc * 130], bf16, name="v_sb")
        attT = pp.tile([128, s], bf16, name="attT")
        wv_sb = pp.tile([128, 8 * CLOC], bf16, name="wv_sb")
        w8_sb = pp.tile([128, 2048], fp8, name="w8_sb")
        wo_sb = pp.tile([128, E], bf16, name="wo_sb")
        bq_sb = pp.tile([128, 3], f32, name="bq_sb")
        id_sb = pp.tile([128, 128], bf16, name="id_sb")
        m_sb = pp.tile([128, 1024], bf16, name="m_sb")

        # ---- const loads. Queues: sync = w8 + fp8 x stream, scalar = bf16
        # x stream + v/out weights. The zero/ones init runs as gpsimd
        # MEMSETs: that engine is otherwise idle, so ~33us of memset
        # executes in parallel with the DMA streams and first proj.
        # qP8's zeroing covers only the never-written regions (tile1 +
        # the off-diagonal tile0 blocks) so phase A's q writes don't
        # queue behind it.
        nc.sync.dma_start(w8_sb[:], w8d[:, :])
        nc.sync.dma_start(bq_sb[:], bqd[:, :])
        nc.scalar.dma_start(wv_sb[:], wvd[:, :])
        nc.scalar.dma_start(id_sb[:], identd[:, :])
        nc.scalar.dma_start(m_sb[:], maskd[:, :])
        nc.scalar.dma_start(wo_sb[:], wod[:, :])
        nc.gpsimd.memset(kT8[:], 0.0)
        nc.gpsimd.memset(v_sb[:], 1.0)
        nc.gpsimd.memset(v8[:], 0.0)
        nc.gpsimd.memset(
            v8.rearrange("p (a c) -> p a c", c=128)[:, :, 64:65], 1.0)
        qP8_b = qP8.rearrange("p (b c) -> p b c", c=1024)
        nc.gpsimd.memset(qP8_b[:, :, 512:1024], 0.0)
        nc.gpsimd.memset(qP8_b[0:64, :, 256:512], 0.0)
        nc.gpsimd.memset(qP8_b[64:128, :, 0:256], 0.0)

        # pools (PSUM: 1 + 2 + 4 + 1 = 8 banks)
        with tc.tile_pool(name="paxt", bufs=3) as paxt, \
             tc.tile_pool(name="pa", bufs=2) as pa, \
             tc.tile_pool(name="ptr", bufs=1, space="PSUM") as ptr, \
             tc.tile_pool(name="pmm", bufs=2, space="PSUM") as pmm, \
             tc.tile_pool(name="psc", bufs=2, space="PSUM") as psc, \
             tc.tile_pool(name="pot", bufs=1, space="PSUM") as pot, \
             tc.tile_pool(name="pbw", bufs=4) as pbw, \
             tc.tile_pool(name="pbwd", bufs=2) as pbwd, \
             tc.tile_pool(name="pbn", bufs=2) as pbn, \
             tc.tile_pool(name="pc", bufs=3) as pc:

            b_queue = []

            def pump_b(k):
                for _ in range(min(k, len(b_queue))):
                    b_queue.pop(0)()

            # ---------------- phase B pieces ----------------
            def qP8_ap(ib):
                return qP8[:, ib * 1024:(ib + 1) * 1024].rearrange(
                    "p (t c) -> p t c", t=2)

            def kT8_ap(jc):
                return kT8[:, jc * 256:(jc + 1) * 256].rearrange(
                    "p (t c) -> p t c", t=2)

            def queue_b(ib):
                st = {}

                def get_ot():
                    if "ot" not in st:
                        st["ot"] = pot.tile([128, 512], f32, tag="ot", name="ot")
                    return st["ot"]

                def t_far_sc(g):
                    scp = psc.tile([128, 1024], f32, tag="sc")
                    for t in range(2):
                        nc.tensor.matmul(
                            scp[:, t * 512:(t + 1) * 512],
                            kT8_ap(2 * g + t), qP8_ap(ib),
                            start=True, stop=True, perf_mode=DR,
                        )
                    # wt8 h-major: [h0: t0(256) t1(256) | h1: t0 t1] so the
                    # AV rhs [2, 256] is contiguous (dual-fp8 ISA rule)
                    wt8 = pbw.tile([128, 1024], fp8, tag="wt8")
                    nc.scalar.activation(
                        wt8.rearrange("p (h t c) -> p t h c", h=2, t=2),
                        scp[:].rearrange("p (t h c) -> p t h c", t=2, h=2),
                        Exp, scale=0.125)
                    st[g] = wt8

                def t_far_av(g, first):
                    wt8 = st.pop(g)
                    ot = get_ot()
                    for h in range(2):
                        nc.tensor.matmul(
                            ot[:, h * 256:(h + 1) * 256],
                            v8[:, g * 512 + h * 256:
                               g * 512 + (h + 1) * 256].rearrange(
                                "p (t c) -> p t c", t=2),
                            wt8[:, h * 512:(h + 1) * 512].rearrange(
                                "p (t c) -> p t c", t=2),
                            start=(first and h == 0), stop=False, perf_mode=DR,
                            skip_group_check=True,
                        )

                def t_diag_sc():
                    scp = psc.tile([128, 1024], f32, tag="sc")
                    for t in range(2):
                        nc.tensor.matmul(
                            scp[:, t * 512:(t + 1) * 512],
                            kT8_ap(2 * ib + t), qP8_ap(ib),
                            start=True, stop=False, perf_mode=DR,
                        )
                        # accumulate the causal mask (-1e4 on masked) so
                        # exp(0.125*(s-240)) ~ 1e-13 ~ 0 (stay inside the ACT Exp table domain)
                        nc.tensor.matmul(
                            scp[:, t * 512:(t + 1) * 512],
                            id_sb[:],
                            m_sb[:, t * 512:(t + 1) * 512],
                            start=False, stop=True, skip_group_check=True,
                        )
                    wt_d = pbwd.tile([128, 1024], bf16, tag="wtd")
                    nc.scalar.activation(wt_d[:], scp[:], Exp, scale=0.125)
                    st["d"] = wt_d

                def t_diag_av(first):
                    wt_d = st.pop("d")
                    ot = get_ot()
                    for t in range(2):
                        jc = 2 * ib + t
                        for h in range(2):
                            nc.tensor.matmul(
                                ot[0:65, h * 256:(h + 1) * 256],
                                v_sb[:, jc * 130 + h * 65:
                                     jc * 130 + (h + 1) * 65],
                                wt_d[:, t * 512 + h * 256:
                                     t * 512 + (h + 1) * 256],
                                start=(first and t == 0 and h == 0),
                                stop=(t == 1 and h == 1),
                                skip_group_check=True,
                            )

                def t_norm():
                    ot = st.pop("ot")
                    onum = pbn.tile([65, 512], f32, tag="onum")
                    nc.vector.tensor_copy(onum[:], ot[0:65, :])
                    # reciprocal_approx_fast (custom DVE op) mishandles a
                    # partition-shifted input on HW: stage the denominator
                    # row to partition 0 first with a plain copy.
                    den = pbn.tile([1, 512], f32, tag="den")
                    nc.vector.tensor_copy(den[0:1, :], onum[64:65, :])
                    rcp = pbn.tile([1, 512], f32, tag="rcp")
                    nc.vector.reciprocal_approx_fast(
                        out=rcp[0:1, :], in_=den[0:1, :])
                    rb = pbn.tile([64, 512], f32, tag="rb")
                    nc.gpsimd.partition_broadcast(rb[:], rcp[0:1, :])
                    for h in range(2):
                        nc.vector.tensor_mul(
                            attT[h * 64:(h + 1) * 64,
                                 ib * 256:(ib + 1) * 256],
                            onum[0:64, h * 256:(h + 1) * 256],
                            rb[:, h * 256:(h + 1) * 256],
                        )

                # software-pipelined: sc(g+1) emitted before av(g) so the
                # PE stays a stage ahead of ACT's exp
                scs = [lambda g=g: t_far_sc(g) for g in range(ib)] + [t_diag_sc]
                avs = [lambda g=g, f=(g == 0): t_far_av(g, f)
                       for g in range(ib)] + [lambda: t_diag_av(ib == 0)]
                thunks = [scs[0]]
                for i in range(1, len(scs)):
                    thunks.append(scs[i])
                    thunks.append(avs[i - 1])
                thunks.append(avs[-1])
                thunks.append(t_norm)
                if ib > 0:
                    thunks.insert(min(2, len(thunks) - 1),
                                  lambda: emit_out_proj(ib - 1, (0,)))
                    thunks.insert(min(5, len(thunks) - 1),
                                  lambda: emit_out_proj(ib - 1, (1,)))
                b_queue.extend(thunks)

            def emit_out_proj(ib, sis=(0, 1)):
                for si in sis:
                    sb2 = 2 * ib + si
                    op = psc.tile([128, 1024], f32, tag="sc")
                    for nh2 in range(2):
                        nc.tensor.matmul(
                            op[:, nh2 * 512:(nh2 + 1) * 512],
                            attT[:, sb2 * 128:(sb2 + 1) * 128],
                            wo_sb[:, nh2 * 512:(nh2 + 1) * 512],
                            start=True, stop=True,
                        )
                    osb = pc.tile([128, 1024], bf16, tag="osb")
                    nc.vector.tensor_copy(osb[:], op[:])
                    nc.gpsimd.dma_start(
                        outp[sb2 * 128:(sb2 + 1) * 128, :], osb[:])

            # ---------------- phase A (interleaved with B) ----------------
            def xT_dma(sb):
                xt = paxt.tile([128, 8 * 512], bf16, tag="xT")
                xt8 = paxt.tile([128, 8 * 512], fp8, tag="xT8")
                nc.sync.dma_start(xt8[:], xT8d[sb * 128:(sb + 1) * 128, :])
                nc.scalar.dma_start(xt[:], xTd[sb * 128:(sb + 1) * 128, :])
                return xt, xt8

            xt_next = xT_dma(0)
            for sb in range(n_sb):
                xT_sb, xT8_sb = xt_next
                if sb + 1 < n_sb:
                    xt_next = xT_dma(sb + 1)
                q = sb + 2  # pump quantum: matches thunk production rate
                vT_t = None
                for t in range(3):
                    mmp = pmm.tile([128, 512], f32, tag="mm")
                    if t < 2:
                        # q/k proj in fp8 DoubleRow: K=256 per pass, so 4
                        # matmuls instead of 8 (cols stream at 1/cyc
                        # regardless of dtype; DR halves the pass count)
                        for pr in range(4):
                            nc.tensor.matmul(
                                mmp[:],
                                w8_sb[:, t * 1024 + pr * 256:
                                      t * 1024 + (pr + 1) * 256].rearrange(
                                    "p (u c) -> p u c", u=2),
                                xT8_sb[:, pr * 1024:(pr + 1) * 1024].rearrange(
                                    "p (u c) -> p u c", u=2),
                                start=(pr == 0), stop=(pr == 3),
                                perf_mode=DR,
                            )
                    else:
                        for ec in range(8):
                            nc.tensor.matmul(
                                mmp[:],
                                wv_sb[:, ec * 128:(ec + 1) * 128],
                                xT_sb[:, ec * 512:(ec + 1) * 512],
                                start=(ec == 0), stop=(ec == 7),
                            )
                    if t == 0:
                        # packed q -> qP8 tile0 halves (fp8), + bias
                        qP8_v = qP8.rearrange("p (b c) -> p b c", c=1024)
                        for h in range(2):
                            dst = qP8_v[h * 64:(h + 1) * 64,
                                        2 * sb:2 * sb + 2,
                                        h * 256:(h + 1) * 256]
                            src = mmp[h * 64:(h + 1) * 64, :].rearrange(
                                "p (b c) -> p b c", c=256)
                            nc.vector.tensor_scalar_add(
                                dst, src, bq_sb[h * 64:(h + 1) * 64, 0:1])
                    elif t == 1:
                        kT8_v = kT8.rearrange("p (b c) -> p b c", c=256)
                        dst = kT8_v[:, 4 * sb:4 * sb + 4, 0:128]
                        src = mmp[:].rearrange("p (b c) -> p b c", c=128)
                        nc.vector.tensor_scalar_add(dst, src, bq_sb[:, 1:2])
                    else:
                        vT_t = pa.tile([128, 512], bf16, tag="vT")
                        nc.vector.tensor_scalar_add(
                            vT_t[:], mmp[:], bq_sb[:, 2:3])
                    pump_b(q)
                for stt in range(4):
                    jc = sb * 4 + stt
                    trv = ptr.tile([128, 128], bf16, tag="tr")
                    nc.tensor.transpose(
                        trv[:],
                        vT_t[:, stt * 128:(stt + 1) * 128],
                        id_sb[:],
                    )
                    g, tt = jc // 2, jc % 2
                    dstd = v_sb[:, jc * 130:(jc + 1) * 130].rearrange(
                        "p (h c) -> p h c", h=2)[:, :, 0:64]
                    src = trv[:].rearrange("p (h c) -> p h c", h=2)
                    nc.vector.tensor_copy(dstd, src)
                    dst8 = v8.rearrange(
                        "p (g h t c) -> p g h t c", h=2, t=2, c=128
                    )[:, g, :, tt, 0:64]
                    nc.vector.tensor_copy(dst8, src)
                    pump_b(q)
                queue_b(2 * sb)
                queue_b(2 * sb + 1)
                pump_b(q)
            while b_queue:
                pump_b(1)
            emit_out_proj(n_ib - 1)
            if debug:
                with tc.tile_pool(name="dbg", bufs=2) as dp:
                    def dump(dst, srct, width, tag):
                        cw = min(2048, width)
                        for c0 in range(0, width, cw):
                            t = dp.tile([128, cw], f32, tag=tag)
                            nc.vector.tensor_copy(t[:], srct[:, c0:c0 + cw])
                            nc.sync.dma_start(dst[:, c0:c0 + cw], t[:])
                    dump(dq, qP8, n_ib * 1024, "d1")
                    dump(dk, kT8, n_jc * 256, "d2")
                    dump(dv, v8, n_g * 512, "d3")
                    dump(da, attT, s, "d4")

    nc.compile()
    return nc


def make_in_maps(x, w_qkv, b_qkv, w_out, b_out, s=S):
    x = np.asarray(x, dtype=np.float32).reshape(s, E)
    w_qkv = np.asarray(w_qkv, dtype=np.float32)
    b_qkv = np.asarray(b_qkv, dtype=np.float32)
    w_out = np.asarray(w_out, dtype=np.float32)
    b_out = np.asarray(b_out, dtype=np.float32)

    # xTt[sb*128+p, e*512+c] = x[sb*512+c, e*128+p]
    x4 = x.reshape(8, 512, 8, 128).transpose(0, 3, 2, 1)  # [sb, p, e, c]
    xTt = np.ascontiguousarray(x4.reshape(1024, 4096))
    xT = xTt.astype(BF)
    xT8 = xTt.astype(F8)
    ident = np.eye(128, dtype=BF)

    # mask tiles: m_t[k, h*256+qq] = 0 if qq >= 128*t + k else -1e4
    qq = np.arange(256)[None, :]
    kk = np.arange(128)[:, None]
    m0 = np.where(qq >= kk, 0.0, -240.0).astype(np.float32)
    m1 = np.where(qq >= 128 + kk, 0.0, -240.0).astype(np.float32)
    masks = np.concatenate([m0, m0, m1, m1], axis=1).astype(BF)

    in_maps = []
    for c in range(N_CORES):
        lo = c * CLOC
        # w8_qk[p, t*1024 + pair*256 + tile*128 + m]
        #   = w_qkv[(2*pair+tile)*128 + p, t*E + lo + m]   (fp8, DR layout)
        w8 = np.zeros((128, 2048), np.float32)
        for t in range(2):
            wt = w_qkv[:, t * E + lo:t * E + lo + CLOC]  # [E, 128]
            for pair in range(4):
                for tl in range(2):
                    src_rows = wt[(2 * pair + tl) * 128:
                                  (2 * pair + tl + 1) * 128, :]
                    w8[:, t * 1024 + pair * 256 + tl * 128:
                       t * 1024 + pair * 256 + (tl + 1) * 128] = src_rows
        w8 = w8.astype(F8)
        # w_vt[p, e*128+m] = w_qkv[e*128+p, 2E+lo+m]
        wv_loc = np.ascontiguousarray(
            w_qkv[:, 2 * E + lo:2 * E + lo + CLOC].reshape(8, 128, 128)
            .transpose(1, 0, 2).reshape(128, E)).astype(BF)
        bq_loc = np.ascontiguousarray(np.stack(
            [b_qkv[lo:lo + CLOC],
             b_qkv[E + lo:E + lo + CLOC],
             b_qkv[2 * E + lo:2 * E + lo + CLOC]], axis=1)).astype(np.float32)
        in_maps.append({
            "xTt": xT,
            "xT8t": xT8,
            "w_vt": wv_loc,
            "w8_qk": w8,
            "bq_loc": bq_loc,
            "w_out_loc": np.ascontiguousarray(
                w_out[lo:lo + CLOC, :]).astype(BF),
            "ident": ident,
            "masks": masks,
        })
    return in_maps


def kernel(x, w_qkv, b_qkv, w_out, b_out, trace=False):
    from concourse.bass_utils import run_bass_kernel_spmd

    if "nc" not in _CACHE:
        _CACHE["nc"] = build_nc()
    nc = _CACHE["nc"]
    in_maps = make_in_maps(x, w_qkv, b_qkv, w_out, b_out)
    last_err = None
    for _attempt in range(2):
        try:
            res = run_bass_kernel_spmd(nc, in_maps, list(range(N_CORES)), trace=trace)
            break
        except Exception as e:  # transient NRT device errors: retry once
            last_err = e
    else:
        raise last_err
    out = np.zeros((S, E), dtype=np.float32)
    for c in range(N_CORES):
        out += np.asarray(res.results[c]["out_p"]).astype(np.float32)
    out += np.asarray(b_out, dtype=np.float32).reshape(1, E)
    _CACHE["last_result"] = res
    return out.reshape(1, S, E)
